# revision 10
# baseline (speedup 1.0000x reference)
"""Two-layer single-head GAT (GATConv x2) on 8 trn2 NeuronCores — v2.

Strategy: 1D node partition across 8 cores by destination node; edges live
with their destination owner; weights replicated.  The dominant cost is the
per-edge dma_gather (Q7 descriptor generation at ~8ns/index), so the design
minimizes gather indices:

  * Self-loops are handled analytically from the resident hoT (own-dst
    features), removing one slot per destination.
  * Destinations are degree-sorted and processed in tiles of 128; each tile
    uses K_t = max in-tile degree slots (single logical slot space).
  * int16 gather indices only reach 32768 rows, so tiles are grouped into
    PHASES: each phase has a compact per-core table holding only the unique
    sources of that phase's edges (< 32768 rows) => a single gather window
    per tile, no lo/hi split padding (~103k slots/core/layer vs 155k).

Per layer, per core (one SPMD launch per layer):
  Stage A (dense): stream hT (bf16) in big DMAs; per 128-row chunk
    matmul -> PSUM [128, 65] = [H = h@W | as = h@(W a_src)];
    copy H (bf16) + as (fp32, packed in bytes 128..131) into 256-byte table
    rows in DRAM.  Per phase a pad row 0 is overwritten with bf16(-3.39e38)
    so padded slots produce exp(score) == 0.
  Own-dst data: ad/as_own ([128,2] per tile) and h_own ([128,64] per tile)
    from hoT matmuls.
  Stage B: per tile, one dma_gather (256B bf16 rows [H|as|junk]) ->
    scores = lrelu(as + ad), p = exp (fused denominator), U = sum p*H,
    plus the analytic self-loop term, out = (U + p_self*h_own)/den + b.

Layer 1 -> layer 2 crosses cores, so the layers run as two SPMD launches;
the host concatenates the layer-1 shard outputs in between.
"""

import sys

sys.path.insert(0, "/opt/trn_rl_repo")

import numpy as np

N = 50000
E = 800000
IN = 128
OUT = 64
C = 8                      # cores
NSH = N // C               # 6250 dsts per core
NTILES = (NSH + 127) // 128  # 49
NSHP = NTILES * 128        # 6272 padded dsts per core
NEG_SLOPE = 0.2
PHASE_LIMIT = 32000        # max unique sources per (core, phase)
PAD_BF16 = -3.3895313892515355e+38  # bf16 0xFF7F; [pad|pad] bytes = fp32 ~-3.4e38


def _build_plan(edge_index):
    """Host-side graph preprocessing shared by both layers.

    Returns dict with: orders, K_t, phase bounds/rows, per-edge compact ids,
    and per-(core,phase) unique source node lists.
    """
    src = np.asarray(edge_index[0], dtype=np.int64)
    dst = np.asarray(edge_index[1], dtype=np.int64)
    core_of = dst // NSH

    deg_all = np.bincount(dst, minlength=N)  # real in-degree (no self loop)
    pos_of = np.empty(N, dtype=np.int64)
    orders = []
    for c in range(C):
        d0 = c * NSH
        order = np.argsort(-deg_all[d0:d0 + NSH], kind="stable")
        pos_of[d0 + order] = np.arange(NSH)
        orders.append(np.concatenate([order + d0, np.full(NSHP - NSH, -1, np.int64)]))

    epos = pos_of[dst]
    etile = epos // 128

    degpt = np.zeros((C, NSHP), np.int64)
    np.add.at(degpt, (core_of, epos), 1)
    K_t = degpt.reshape(C, NTILES, 128).max(axis=(0, 2))
    K_t = np.maximum(K_t, 1)

    # slot of each edge within its (core, dst)
    okey = np.lexsort((epos, core_of))
    gid = core_of[okey] * NSHP + epos[okey]
    first = np.r_[True, gid[1:] != gid[:-1]]
    lin = np.arange(len(gid))
    start = np.maximum.accumulate(np.where(first, lin, 0))
    slot = np.empty(E, np.int64)
    slot[okey] = lin - start

    # phase boundaries (shared across cores): greedy largest ranges with
    # per-core unique source count <= PHASE_LIMIT
    bounds = []
    t0 = 0
    while t0 < NTILES:
        t1 = t0 + 1
        while t1 < NTILES:
            ok = True
            for c in range(C):
                m = (core_of == c) & (etile >= t0) & (etile <= t1)
                if len(np.unique(src[m])) > PHASE_LIMIT:
                    ok = False
                    break
            if not ok:
                break
            t1 += 1
        bounds.append((t0, t1))
        t0 = t1

    compact = np.zeros(E, np.int64)   # 1-based id within the edge's phase window
    uniq_lists = []                   # [phase][core] -> node ids
    R_list, row0_list = [], []
    row0 = 0
    phase_of_tile = np.zeros(NTILES, np.int64)
    for p, (ta, tb) in enumerate(bounds):
        phase_of_tile[ta:tb] = p
        us = []
        umax = 0
        for c in range(C):
            m = (core_of == c) & (etile >= ta) & (etile < tb)
            uniq, inv = np.unique(src[m], return_inverse=True)
            compact[m] = inv + 1
            us.append(uniq)
            umax = max(umax, len(uniq))
        uniq_lists.append(us)
        # round to 2048 (= GRP chunks of 128) so the stage-A group write
        # permutation (see _phys_row) stays within a phase
        R = ((umax + 1 + 2047) // 2048) * 2048
        assert R <= 32768
        R_list.append(R)
        row0_list.append(row0)
        row0 += R
    tot = row0

    return dict(orders=orders, K_t=K_t, bounds=bounds, R_list=R_list,
                row0_list=row0_list, tot=tot, uniq_lists=uniq_lists,
                phase_of_tile=phase_of_tile, core_of=core_of, epos=epos,
                etile=etile, slot=slot, compact=compact)


GRP = 16  # stage-A chunks (of 128 rows) per group DMA


def _phys_row(lp):
    """Logical phase-relative row -> physical table row.

    Stage A writes a [128, GRP, 128] staging tile to 2048 consecutive table
    rows; the DMA pairs elements in AP order, which lands logical row
    128*j + p (chunk j, partition p) at physical row p*GRP + j within the
    group. Indices must address physical rows.
    """
    lp = np.asarray(lp)
    base = (lp // (128 * GRP)) * (128 * GRP)
    off = lp - base
    return base + (off % 128) * GRP + off // 128


def _wrap_idx(arr):
    """[K,128] slot-major idx array -> [128, 8K] wrapped+replicated int16."""
    flat = arr.reshape(-1)                       # i = k*128 + p
    w = flat.reshape(-1, 16).T                   # [16, NI/16]
    return np.tile(w, (8, 1)).astype(np.int16)


def _build_idx_tensor(plan):
    """Per-core [128, IDXCOLS] int16 idx tensors + per-tile offsets."""
    K_t = plan["K_t"]
    offs = []
    off = 0
    for t in range(NTILES):
        offs.append(off)
        off += 8 * int(K_t[t])
    idxcols = off
    out = np.zeros((C, 128, idxcols), np.int16)
    core_of, epos, etile = plan["core_of"], plan["etile"], plan["etile"]
    epos = plan["epos"]
    slot, compact = plan["slot"], plan["compact"]
    for c in range(C):
        mc = core_of == c
        for t in range(NTILES):
            m = mc & (etile == t)
            arr = np.zeros((int(K_t[t]), 128), np.int64)   # pad -> row 0
            arr[slot[m], epos[m] % 128] = _phys_row(compact[m])
            out[c, :, offs[t]:offs[t] + 8 * int(K_t[t])] = _wrap_idx(arr)
    return out, offs, idxcols


def _build_launch(kdim, plan, offs, idxcols, dbg_tile=None):
    """One SPMD launch: Stage A (compact tables) + Stage B (49 dst tiles)."""
    import concourse.bacc as bacc
    import concourse.mybir as mybir
    from concourse.tile import TileContext

    f32 = mybir.dt.float32
    bf16 = mybir.dt.bfloat16
    K_t = plan["K_t"]
    R_list, row0_list = plan["R_list"], plan["row0_list"]
    phase_of = plan["phase_of_tile"]
    tot = plan["tot"]
    nchunk = tot // 128

    nc = bacc.Bacc(None, target_bir_lowering=False, debug=True)
    hT = nc.declare_dram_parameter("hT", [kdim, tot], bf16, isOutput=False)
    hoT = nc.declare_dram_parameter("hoT", [kdim, NSHP], bf16, isOutput=False)
    wse = nc.declare_dram_parameter("wse", [kdim, 65], bf16, isOutput=False)
    wad2 = nc.declare_dram_parameter("wad2", [kdim, 2], bf16, isOutput=False)
    rb = nc.declare_dram_parameter("rb", [128, 64], f32, isOutput=False)
    idx = nc.declare_dram_parameter("idx", [128, idxcols], mybir.dt.int16, isOutput=False)
    outp = nc.declare_dram_parameter("outp", [NSHP, 64], f32, isOutput=True)
    tabl = nc.dram_tensor("tabl", [tot, 128], bf16)
    assert tot % (128 * GRP) == 0
    if dbg_tile is not None:
        kdbg = int(K_t[dbg_tile])
        dbg_tg = nc.declare_dram_parameter("dbg_tg", [128, kdbg, 128], bf16, isOutput=True)
        dbg_sc = nc.declare_dram_parameter("dbg_sc", [128, 3 * kdbg + 200], f32, isOutput=True)

    with TileContext(nc) as tc:
        with (
            tc.tile_pool(name="const", bufs=1) as cpool,
            tc.tile_pool(name="io", bufs=2) as io,
            tc.tile_pool(name="acopy", bufs=2) as acopy,
            tc.tile_pool(name="work", bufs=3) as work,
            tc.tile_pool(name="small", bufs=2) as small,
            tc.tile_pool(name="psA", bufs=3, space="PSUM") as psA,
            tc.tile_pool(name="psB", bufs=2, space="PSUM") as psB,
            tc.tile_pool(name="psH", bufs=2, space="PSUM") as psH,
        ):
            wse_sb = cpool.tile([kdim, 65], bf16)
            nc.sync.dma_start(out=wse_sb[:, :], in_=wse[:, :])
            wad2_sb = cpool.tile([kdim, 2], bf16)
            nc.sync.dma_start(out=wad2_sb[:, :], in_=wad2[:, :])
            rb_sb = cpool.tile([128, 64], f32)
            nc.sync.dma_start(out=rb_sb[:, :], in_=rb[:, :])
            idx_sb = cpool.tile([128, idxcols], mybir.dt.int16)
            nc.sync.dma_start(out=idx_sb[:, :], in_=idx[:, :])
            hoT_sb = cpool.tile([kdim, NSHP], bf16)
            nc.sync.dma_start(out=hoT_sb[:, :], in_=hoT[:, :])
            padrow = cpool.tile([1, 128], bf16)
            nc.vector.memset(padrow[:, :], PAD_BF16)
            adas_sb = cpool.tile([128, NTILES, 2], f32)
            ho_all = cpool.tile([128, NTILES, 64], f32)

            # Stage A: table rows [H(64 bf16) | as(fp32) | junk] per chunk group
            for g in range((nchunk + GRP - 1) // GRP):
                ch0 = g * GRP
                n = min(GRP, nchunk - ch0)
                hg = io.tile([kdim, n * 128], bf16, tag="hg")
                nc.sync.dma_start(out=hg[:, :], in_=hT[:, 128 * ch0:128 * (ch0 + n)])
                st = acopy.tile([128, n, 128], bf16, tag="st")
                for j in range(n):
                    ps = psA.tile([128, 65], f32)
                    nc.tensor.matmul(ps[:, :], hg[:, 128 * j:128 * (j + 1)],
                                     wse_sb[:, :], start=True, stop=True)
                    nc.scalar.copy(st[:, j, 0:64], ps[:, 0:64])
                    nc.vector.tensor_copy(st[:, j, 64:66].bitcast(f32), ps[:, 64:65])
                nc.sync.dma_start(out=tabl[128 * ch0:128 * (ch0 + n), :], in_=st[:, :, :])

            # per-phase pad rows AFTER their chunk writes
            for row0 in row0_list:
                nc.sync.dma_start(out=tabl[row0:row0 + 1, :], in_=padrow[0:1, :])

            # own-dst data: [ad | as_own] and h_own per tile
            for t in range(NTILES):
                hsl = hoT_sb[:, 128 * t:128 * (t + 1)]
                ps2 = psB.tile([128, 2], f32)
                nc.tensor.matmul(ps2[:, :], hsl, wad2_sb[:, :], start=True, stop=True)
                nc.scalar.copy(adas_sb[:, t, :], ps2[:, :])
                psh = psH.tile([128, 64], f32)
                nc.tensor.matmul(psh[:, :], hsl, wse_sb[:, 0:64], start=True, stop=True)
                nc.scalar.copy(ho_all[:, t, :], psh[:, :])

            # Stage B: one 128-dst tile at a time
            for t in range(NTILES):
                k = int(K_t[t])
                p = int(phase_of[t])
                row0, R = row0_list[p], R_list[p]
                tg = work.tile([128, k, 128], bf16, tag="tg")
                nc.gpsimd.dma_gather(tg[:, :, :], tabl[row0:row0 + R, :],
                                     idx_sb[:, offs[t]:offs[t] + 8 * k],
                                     128 * k, 128 * k, 128, single_packet=False)
                as_t = small.tile([128, k, 1], f32, tag="as")
                nc.vector.tensor_copy(as_t[:, :, :], tg[:, :, 64:66].bitcast(f32))
                z_t = small.tile([128, k], f32, tag="z")
                nc.vector.tensor_scalar(z_t[:, :], as_t[:, :, :].squeeze(2),
                                        adas_sb[:, t, 0:1], None, mybir.AluOpType.add)
                s_t = small.tile([128, k], f32, tag="s")
                nc.vector.scalar_tensor_tensor(s_t[:, :], z_t[:, :], NEG_SLOPE, z_t[:, :],
                                               mybir.AluOpType.mult, mybir.AluOpType.max)
                p_t = small.tile([128, k], bf16, tag="p")
                den = small.tile([128, 1], f32, tag="den")
                nc.scalar.activation(p_t[:, :], s_t[:, :], mybir.ActivationFunctionType.Exp,
                                     accum_out=den[:, :])
                # self-loop score
                z2 = small.tile([128, 1], f32, tag="z2")
                nc.vector.tensor_tensor(z2[:, :], adas_sb[:, t, 1:2], adas_sb[:, t, 0:1],
                                        mybir.AluOpType.add)
                s2 = small.tile([128, 1], f32, tag="s2")
                nc.vector.scalar_tensor_tensor(s2[:, :], z2[:, :], NEG_SLOPE, z2[:, :],
                                               mybir.AluOpType.mult, mybir.AluOpType.max)
                p2 = small.tile([128, 1], f32, tag="p2")
                nc.scalar.activation(p2[:, :], s2[:, :], mybir.ActivationFunctionType.Exp)
                den2 = small.tile([128, 1], f32, tag="den2")
                nc.vector.tensor_tensor(den2[:, :], den[:, :], p2[:, :], mybir.AluOpType.add)
                # weighted aggregation
                pt = work.tile([128, k, 64], bf16, tag="pt")
                p_b = p_t[:, :].unsqueeze(2).broadcast_to([128, k, 64])
                nc.vector.tensor_tensor(pt[:, :, :], tg[:, :, 0:64], p_b, mybir.AluOpType.mult)
                u = small.tile([128, 64], f32, tag="u")
                nc.vector.tensor_reduce(u[:, :], pt[:, :, :].transpose([0, 2, 1]),
                                        mybir.AxisListType.X, mybir.AluOpType.add)
                u2 = small.tile([128, 64], f32, tag="u2")
                nc.vector.scalar_tensor_tensor(u2[:, :], ho_all[:, t, :], p2[:, :], u[:, :],
                                               mybir.AluOpType.mult, mybir.AluOpType.add)
                rd = small.tile([128, 1], f32, tag="rd")
                nc.vector.reciprocal(rd[:, :], den2[:, :])
                o = small.tile([128, 64], f32, tag="o")
                nc.vector.scalar_tensor_tensor(o[:, :], u2[:, :], rd[:, :], rb_sb[:, :],
                                               mybir.AluOpType.mult, mybir.AluOpType.add)
                nc.sync.dma_start(out=outp[128 * t:128 * (t + 1), :], in_=o[:, :])
                if dbg_tile == t:
                    nc.sync.dma_start(out=dbg_tg[:, :, :], in_=tg[:, :, :])
                    dsc = small.tile([128, 3 * k + 200], f32, tag="dsc")
                    nc.vector.tensor_copy(dsc[:, 0:k], as_t[:, :, :].squeeze(2))
                    nc.vector.tensor_copy(dsc[:, k:2 * k], s_t[:, :])
                    nc.vector.tensor_copy(dsc[:, 2 * k:3 * k], p_t[:, :])
                    base = 3 * k
                    nc.vector.tensor_copy(dsc[:, base:base + 1], den[:, :])
                    nc.vector.tensor_copy(dsc[:, base + 1:base + 2], den2[:, :])
                    nc.vector.tensor_copy(dsc[:, base + 2:base + 3], p2[:, :])
                    nc.vector.tensor_copy(dsc[:, base + 3:base + 4], z2[:, :])
                    nc.vector.tensor_copy(dsc[:, base + 4:base + 68], u[:, :])
                    nc.vector.tensor_copy(dsc[:, base + 68:base + 132], u2[:, :])
                    nc.vector.tensor_copy(dsc[:, base + 132:base + 134], adas_sb[:, t, :])
                    nc.vector.tensor_copy(dsc[:, base + 134:base + 198], ho_all[:, t, :])
                    nc.sync.dma_start(out=dbg_sc[:, :], in_=dsc[:, :])

    nc.compile()
    return nc


def _bf16(a):
    import concourse.mybir as mybir
    return np.asarray(a).astype(mybir.dt.np(mybir.dt.bfloat16))


def _build_hT(plan, hT_full_f32):
    """Per-core compact hT: [kdim, tot] with per-phase [pad | uniq sources]."""
    kdim = hT_full_f32.shape[0]
    tot = plan["tot"]
    out = []
    for c in range(C):
        hc = np.zeros((kdim, tot), np.float32)
        for p, row0 in enumerate(plan["row0_list"]):
            uniq = plan["uniq_lists"][p][c]
            hc[:, row0 + 1:row0 + 1 + len(uniq)] = hT_full_f32[:, uniq]
        out.append(_bf16(hc))
    return out


def _build_hoT(plan, hT_full_f32):
    kdim = hT_full_f32.shape[0]
    out = []
    for c in range(C):
        own = plan["orders"][c]
        ho = np.zeros((kdim, NSHP), np.float32)
        real = own >= 0
        ho[:, real] = hT_full_f32[:, own[real]]
        out.append(_bf16(ho))
    return out


LAST = {}


def kernel(x, edge_index, W1, a_src1, a_dst1, b1, W2, a_src2, a_dst2, b2):
    from concourse.bass_utils import run_bass_kernel_spmd

    x = np.asarray(x, np.float32)
    edge_index = np.asarray(edge_index)
    W1 = np.asarray(W1, np.float64); a_src1 = np.asarray(a_src1, np.float64)
    a_dst1 = np.asarray(a_dst1, np.float64); b1 = np.asarray(b1, np.float32)
    W2 = np.asarray(W2, np.float64); a_src2 = np.asarray(a_src2, np.float64)
    a_dst2 = np.asarray(a_dst2, np.float64); b2 = np.asarray(b2, np.float32)

    plan = _build_plan(edge_index)
    idx, offs, idxcols = _build_idx_tensor(plan)

    nc1 = _build_launch(IN, plan, offs, idxcols)
    nc2 = _build_launch(OUT, plan, offs, idxcols)

    w1se = _bf16(np.concatenate([W1, (W1 @ a_src1)[:, None]], 1))
    w2se = _bf16(np.concatenate([W2, (W2 @ a_src2)[:, None]], 1))
    w1ad2 = _bf16(np.stack([W1 @ a_dst1, W1 @ a_src1], 1))
    w2ad2 = _bf16(np.stack([W2 @ a_dst2, W2 @ a_src2], 1))
    rb1 = np.tile(b1, (128, 1)).astype(np.float32)
    rb2 = np.tile(b2, (128, 1)).astype(np.float32)

    xT = np.ascontiguousarray(x.T)
    hTs1 = _build_hT(plan, xT)
    hoTs1 = _build_hoT(plan, xT)
    in_maps1 = [{"hT": hTs1[c], "hoT": hoTs1[c], "wse": w1se, "wad2": w1ad2,
                 "rb": rb1, "idx": idx[c]} for c in range(C)]

    res1 = run_bass_kernel_spmd(nc1, in_maps1, core_ids=list(range(C)))
    LAST["res1"] = res1

    # assemble full node-indexed h2
    h2 = np.zeros((N, OUT), np.float32)
    for c in range(C):
        sh = np.asarray(res1.results[c]["outp"])
        own = plan["orders"][c]
        real = own >= 0
        h2[own[real]] = sh[real]
    h2T = np.ascontiguousarray(h2.T)

    hTs2 = _build_hT(plan, h2T)
    hoTs2 = _build_hoT(plan, h2T)
    in_maps2 = [{"hT": hTs2[c], "hoT": hoTs2[c], "wse": w2se, "wad2": w2ad2,
                 "rb": rb2, "idx": idx[c]} for c in range(C)]

    res2 = run_bass_kernel_spmd(nc2, in_maps2, core_ids=list(range(C)))
    LAST["res2"] = res2

    out = np.empty((N, OUT), np.float32)
    for c in range(C):
        sh = np.asarray(res2.results[c]["outp"])
        own = plan["orders"][c]
        real = own >= 0
        out[own[real]] = sh[real]
    return out


# revision 19
# speedup vs baseline: 1.0890x; 1.0890x over previous
"""Two-layer single-head GAT (GATConv x2) on 8 trn2 NeuronCores — v2.

Strategy: 1D node partition across 8 cores by destination node; edges live
with their destination owner; weights replicated.  The dominant cost is the
per-edge dma_gather (Q7 descriptor generation at ~8ns/index), so the design
minimizes gather indices:

  * Self-loops are handled analytically from the resident hoT (own-dst
    features), removing one slot per destination.
  * Destinations are degree-sorted and processed in tiles of 128; each tile
    uses K_t = max in-tile degree slots (single logical slot space).
  * int16 gather indices only reach 32768 rows, so tiles are grouped into
    PHASES: each phase has a compact per-core table holding only the unique
    sources of that phase's edges (< 32768 rows) => a single gather window
    per tile, no lo/hi split padding (~103k slots/core/layer vs 155k).

Per layer, per core (one SPMD launch per layer):
  Stage A (dense): stream hT (bf16) in big DMAs; per 128-row chunk
    matmul -> PSUM [128, 65] = [H = h@W | as = h@(W a_src)];
    copy H (bf16) + as (fp32, packed in bytes 128..131) into 256-byte table
    rows in DRAM.  Per phase a pad row 0 is overwritten with bf16(-3.39e38)
    so padded slots produce exp(score) == 0.
  Own-dst data: ad/as_own ([128,2] per tile) and h_own ([128,64] per tile)
    from hoT matmuls.
  Stage B: per tile, one dma_gather (256B bf16 rows [H|as|junk]) ->
    scores = lrelu(as + ad), p = exp (fused denominator), U = sum p*H,
    plus the analytic self-loop term, out = (U + p_self*h_own)/den + b.

Layer 1 -> layer 2 crosses cores, so the layers run as two SPMD launches;
the host concatenates the layer-1 shard outputs in between.
"""

import sys

sys.path.insert(0, "/opt/trn_rl_repo")

import numpy as np

N = 50000
E = 800000
IN = 128
OUT = 64
C = 8                      # cores
NSH = N // C               # 6250 dsts per core
NTILES = (NSH + 127) // 128  # 49
NSHP = NTILES * 128        # 6272 padded dsts per core
NEG_SLOPE = 0.2
PHASE_LIMIT = 32000        # max unique sources per (core, phase)
# pad-row element: sum of 64 of these stays finite (~-6.4e37) and exp -> 0
PAD_BF16 = -1.0e36


def _build_plan(edge_index):
    """Host-side graph preprocessing shared by both layers.

    Returns dict with: orders, K_t, phase bounds/rows, per-edge compact ids,
    and per-(core,phase) unique source node lists.
    """
    src = np.asarray(edge_index[0], dtype=np.int64)
    dst = np.asarray(edge_index[1], dtype=np.int64)
    core_of = dst // NSH

    deg_all = np.bincount(dst, minlength=N)  # real in-degree (no self loop)
    pos_of = np.empty(N, dtype=np.int64)
    orders = []
    for c in range(C):
        d0 = c * NSH
        order = np.argsort(-deg_all[d0:d0 + NSH], kind="stable")
        pos_of[d0 + order] = np.arange(NSH)
        orders.append(np.concatenate([order + d0, np.full(NSHP - NSH, -1, np.int64)]))

    epos = pos_of[dst]
    etile = epos // 128

    degpt = np.zeros((C, NSHP), np.int64)
    np.add.at(degpt, (core_of, epos), 1)
    K_t = degpt.reshape(C, NTILES, 128).max(axis=(0, 2))
    K_t = np.maximum(K_t, 1)

    # slot of each edge within its (core, dst)
    okey = np.lexsort((epos, core_of))
    gid = core_of[okey] * NSHP + epos[okey]
    first = np.r_[True, gid[1:] != gid[:-1]]
    lin = np.arange(len(gid))
    start = np.maximum.accumulate(np.where(first, lin, 0))
    slot = np.empty(E, np.int64)
    slot[okey] = lin - start

    # phase boundaries (shared across cores): greedy largest ranges with
    # per-core unique source count <= PHASE_LIMIT
    bounds = []
    t0 = 0
    while t0 < NTILES:
        t1 = t0 + 1
        while t1 < NTILES:
            ok = True
            for c in range(C):
                m = (core_of == c) & (etile >= t0) & (etile <= t1)
                if len(np.unique(src[m])) > PHASE_LIMIT:
                    ok = False
                    break
            if not ok:
                break
            t1 += 1
        bounds.append((t0, t1))
        t0 = t1

    compact = np.zeros(E, np.int64)   # 1-based id within the edge's phase window
    uniq_lists = []                   # [phase][core] -> node ids
    R_list, row0_list = [], []
    row0 = 0
    phase_of_tile = np.zeros(NTILES, np.int64)
    for p, (ta, tb) in enumerate(bounds):
        phase_of_tile[ta:tb] = p
        us = []
        umax = 0
        for c in range(C):
            m = (core_of == c) & (etile >= ta) & (etile < tb)
            uniq, inv = np.unique(src[m], return_inverse=True)
            compact[m] = inv + 1
            us.append(uniq)
            umax = max(umax, len(uniq))
        uniq_lists.append(us)
        # round to 2048 (= GRP chunks of 128) so the stage-A group write
        # permutation (see _phys_row) stays within a phase
        R = ((umax + 1 + 2047) // 2048) * 2048
        assert R <= 32768
        R_list.append(R)
        row0_list.append(row0)
        row0 += R
    tot = row0

    return dict(orders=orders, K_t=K_t, bounds=bounds, R_list=R_list,
                row0_list=row0_list, tot=tot, uniq_lists=uniq_lists,
                phase_of_tile=phase_of_tile, core_of=core_of, epos=epos,
                etile=etile, slot=slot, compact=compact)


GRP = 16  # stage-A chunks (of 128 rows) per group DMA


def _phys_row(lp):
    """Logical phase-relative row -> physical table row.

    Stage A writes a [128, GRP, 128] staging tile to 2048 consecutive table
    rows; the DMA pairs elements in AP order, which lands logical row
    128*j + p (chunk j, partition p) at physical row p*GRP + j within the
    group. Indices must address physical rows.
    """
    lp = np.asarray(lp)
    base = (lp // (128 * GRP)) * (128 * GRP)
    off = lp - base
    return base + (off % 128) * GRP + off // 128


def _wrap_idx(arr):
    """[K,128] slot-major idx array -> [128, 8K] wrapped+replicated int16."""
    flat = arr.reshape(-1)                       # i = k*128 + p
    w = flat.reshape(-1, 16).T                   # [16, NI/16]
    return np.tile(w, (8, 1)).astype(np.int16)


def _build_idx_tensor(plan):
    """Per-core [128, IDXCOLS] int16 idx tensors + per-tile offsets."""
    K_t = plan["K_t"]
    offs = []
    off = 0
    for t in range(NTILES):
        offs.append(off)
        off += 8 * int(K_t[t])
    idxcols = off
    out = np.zeros((C, 128, idxcols), np.int16)
    core_of, epos, etile = plan["core_of"], plan["etile"], plan["etile"]
    epos = plan["epos"]
    slot, compact = plan["slot"], plan["compact"]
    for c in range(C):
        mc = core_of == c
        for t in range(NTILES):
            m = mc & (etile == t)
            arr = np.zeros((int(K_t[t]), 128), np.int64)   # pad -> row 0
            arr[slot[m], epos[m] % 128] = _phys_row(compact[m])
            out[c, :, offs[t]:offs[t] + 8 * int(K_t[t])] = _wrap_idx(arr)
    return out, offs, idxcols


def _build_launch(kdim, plan, offs, idxcols, dbg_tile=None):
    """One SPMD launch: Stage A (compact tables) + Stage B (49 dst tiles)."""
    import concourse.bacc as bacc
    import concourse.mybir as mybir
    from concourse.tile import TileContext

    f32 = mybir.dt.float32
    bf16 = mybir.dt.bfloat16
    K_t = plan["K_t"]
    R_list, row0_list = plan["R_list"], plan["row0_list"]
    phase_of = plan["phase_of_tile"]
    tot = plan["tot"]
    nchunk = tot // 128

    nc = bacc.Bacc(None, target_bir_lowering=False, debug=True)
    hT = nc.declare_dram_parameter("hT", [kdim, tot], bf16, isOutput=False)
    hoT = nc.declare_dram_parameter("hoT", [kdim, NSHP], bf16, isOutput=False)
    wse = nc.declare_dram_parameter("wse", [kdim, 128], bf16, isOutput=False)
    wad2 = nc.declare_dram_parameter("wad2", [kdim, 2], bf16, isOutput=False)
    rb = nc.declare_dram_parameter("rb", [128, 64], f32, isOutput=False)
    idx = nc.declare_dram_parameter("idx", [128, idxcols], mybir.dt.int16, isOutput=False)
    outp = nc.declare_dram_parameter("outp", [NSHP, 64], f32, isOutput=True)
    tabl = nc.dram_tensor("tabl", [tot, 128], bf16)
    assert tot % (128 * GRP) == 0
    if dbg_tile is not None:
        kdbg = int(K_t[dbg_tile])
        dbg_tg = nc.declare_dram_parameter("dbg_tg", [128, kdbg, 128], bf16, isOutput=True)
        dbg_sc = nc.declare_dram_parameter("dbg_sc", [128, 3 * kdbg + 200], f32, isOutput=True)

    with TileContext(nc) as tc:
        with (
            tc.tile_pool(name="const", bufs=1) as cpool,
            tc.tile_pool(name="io", bufs=2) as io,
            tc.tile_pool(name="acopy", bufs=2) as acopy,
            tc.tile_pool(name="work", bufs=4) as work,
            tc.tile_pool(name="small", bufs=2) as small,
            tc.tile_pool(name="psA", bufs=3, space="PSUM") as psA,
            tc.tile_pool(name="psB", bufs=2, space="PSUM") as psB,
            tc.tile_pool(name="psH", bufs=2, space="PSUM") as psH,
        ):
            wse_sb = cpool.tile([kdim, 128], bf16)
            nc.sync.dma_start(out=wse_sb[:, :], in_=wse[:, :])
            wad2_sb = cpool.tile([kdim, 2], bf16)
            nc.sync.dma_start(out=wad2_sb[:, :], in_=wad2[:, :])
            rb_sb = cpool.tile([128, 64], f32)
            nc.sync.dma_start(out=rb_sb[:, :], in_=rb[:, :])
            idx_sb = cpool.tile([128, idxcols], mybir.dt.int16)
            nc.sync.dma_start(out=idx_sb[:, :], in_=idx[:, :])
            hoT_sb = cpool.tile([kdim, NSHP], bf16)
            nc.sync.dma_start(out=hoT_sb[:, :], in_=hoT[:, :])
            padrow = cpool.tile([1, 128], bf16)
            nc.vector.memset(padrow[:, :], PAD_BF16)
            adas_sb = cpool.tile([128, NTILES, 2], f32)
            ho_all = cpool.tile([128, NTILES, 64], f32)

            # Stage A: table rows [S = h@(W*a_src) (64) | H = h@W (64)] bf16
            for g in range(nchunk // GRP):
                ch0 = g * GRP
                hg = io.tile([kdim, GRP * 128], bf16, tag="hg")
                nc.sync.dma_start(out=hg[:, :], in_=hT[:, 128 * ch0:128 * (ch0 + GRP)])
                st = acopy.tile([128, GRP, 128], bf16, tag="st")
                for j4 in range(GRP // 4):
                    ps = psA.tile([128, 4, 128], f32)
                    for jj in range(4):
                        j = j4 * 4 + jj
                        nc.tensor.matmul(ps[:, jj, :], hg[:, 128 * j:128 * (j + 1)],
                                         wse_sb[:, :], start=True, stop=True)
                    nc.scalar.copy(st[:, 4 * j4:4 * (j4 + 1), :], ps[:, :, :])
                nc.sync.dma_start(out=tabl[128 * ch0:128 * (ch0 + GRP), :], in_=st[:, :, :])

            # per-phase pad rows AFTER their chunk writes
            for row0 in row0_list:
                nc.sync.dma_start(out=tabl[row0:row0 + 1, :], in_=padrow[0:1, :])

            # own-dst data: [ad | as_own] and h_own per tile
            for t in range(NTILES):
                hsl = hoT_sb[:, 128 * t:128 * (t + 1)]
                ps2 = psB.tile([128, 2], f32)
                nc.tensor.matmul(ps2[:, :], hsl, wad2_sb[:, :], start=True, stop=True)
                nc.scalar.copy(adas_sb[:, t, :], ps2[:, :])
                psh = psH.tile([128, 64], f32)
                nc.tensor.matmul(psh[:, :], hsl, wse_sb[:, 64:128], start=True, stop=True)
                nc.scalar.copy(ho_all[:, t, :], psh[:, :])

            # Stage B: one 128-dst tile at a time
            for t in range(NTILES):
                k = int(K_t[t])
                p = int(phase_of[t])
                row0, R = row0_list[p], R_list[p]
                tg = work.tile([128, k, 128], bf16, tag="tg")
                nc.gpsimd.dma_gather(tg[:, :, :], tabl[row0:row0 + R, :],
                                     idx_sb[:, offs[t]:offs[t] + 8 * k],
                                     128 * k, 128 * k, 128, single_packet=False)
                as_t = small.tile([128, k], f32, tag="as")
                nc.vector.tensor_reduce(as_t[:, :], tg[:, :, 0:64], mybir.AxisListType.X,
                                        mybir.AluOpType.add)
                z_t = small.tile([128, k], f32, tag="z")
                nc.vector.tensor_scalar(z_t[:, :], as_t[:, :], adas_sb[:, t, 0:1], None,
                                        mybir.AluOpType.add)
                s_t = small.tile([128, k], f32, tag="s")
                nc.vector.scalar_tensor_tensor(s_t[:, :], z_t[:, :], NEG_SLOPE, z_t[:, :],
                                               mybir.AluOpType.mult, mybir.AluOpType.max)
                p_t = small.tile([128, k], bf16, tag="p")
                den = small.tile([128, 1], f32, tag="den")
                nc.scalar.activation(p_t[:, :], s_t[:, :], mybir.ActivationFunctionType.Exp,
                                     accum_out=den[:, :])
                # self-loop score: s2 = lrelu(as_own + ad)
                z2 = small.tile([128, 1], f32, tag="z2")
                nc.vector.tensor_tensor(z2[:, :], adas_sb[:, t, 1:2], adas_sb[:, t, 0:1],
                                        mybir.AluOpType.add)
                s2 = small.tile([128, 1], f32, tag="s2")
                nc.vector.scalar_tensor_tensor(s2[:, :], z2[:, :], NEG_SLOPE, z2[:, :],
                                               mybir.AluOpType.mult, mybir.AluOpType.max)
                p2 = small.tile([128, 1], f32, tag="p2")
                nc.scalar.activation(p2[:, :], s2[:, :], mybir.ActivationFunctionType.Exp)
                den2 = small.tile([128, 1], f32, tag="den2")
                nc.vector.tensor_tensor(den2[:, :], den[:, :], p2[:, :], mybir.AluOpType.add)
                # weighted aggregation over the H half
                pt = work.tile([128, k, 64], bf16, tag="pt")
                p_b = p_t[:, :].unsqueeze(2).broadcast_to([128, k, 64])
                nc.vector.tensor_tensor(pt[:, :, :], tg[:, :, 64:128], p_b, mybir.AluOpType.mult)
                u = small.tile([128, 64], f32, tag="u")
                nc.vector.tensor_reduce(u[:, :], pt[:, :, :].transpose([0, 2, 1]),
                                        mybir.AxisListType.X, mybir.AluOpType.add)
                ho_p = small.tile([128, 64], f32, tag="ho_p")
                nc.scalar.activation(ho_p[:, :], ho_all[:, t, :],
                                     mybir.ActivationFunctionType.Copy, scale=p2[:, :])
                u2 = small.tile([128, 64], f32, tag="u2")
                nc.vector.tensor_tensor(u2[:, :], ho_p[:, :], u[:, :], mybir.AluOpType.add)
                rd = small.tile([128, 1], f32, tag="rd")
                nc.vector.reciprocal(rd[:, :], den2[:, :])
                o1 = small.tile([128, 64], f32, tag="o1")
                nc.scalar.activation(o1[:, :], u2[:, :],
                                     mybir.ActivationFunctionType.Copy, scale=rd[:, :])
                o = small.tile([128, 64], f32, tag="o")
                nc.vector.tensor_tensor(o[:, :], o1[:, :], rb_sb[:, :], mybir.AluOpType.add)
                nc.sync.dma_start(out=outp[128 * t:128 * (t + 1), :], in_=o[:, :])
                if dbg_tile == t:
                    nc.sync.dma_start(out=dbg_tg[:, :, :], in_=tg[:, :, :])
                    dsc = small.tile([128, 3 * k + 200], f32, tag="dsc")
                    nc.vector.tensor_copy(dsc[:, 0:k], as_t[:, :])
                    nc.vector.tensor_copy(dsc[:, k:2 * k], s_t[:, :])
                    nc.vector.tensor_copy(dsc[:, 2 * k:3 * k], p_t[:, :])
                    base = 3 * k
                    nc.vector.tensor_copy(dsc[:, base:base + 1], den[:, :])
                    nc.vector.tensor_copy(dsc[:, base + 1:base + 2], den2[:, :])
                    nc.vector.tensor_copy(dsc[:, base + 2:base + 3], p2[:, :])
                    nc.vector.tensor_copy(dsc[:, base + 3:base + 4], s2[:, :])
                    nc.vector.tensor_copy(dsc[:, base + 4:base + 68], u[:, :])
                    nc.vector.tensor_copy(dsc[:, base + 68:base + 132], u2[:, :])
                    nc.vector.tensor_copy(dsc[:, base + 132:base + 134], adas_sb[:, t, :])
                    nc.vector.tensor_copy(dsc[:, base + 134:base + 198], ho_all[:, t, :])
                    nc.sync.dma_start(out=dbg_sc[:, :], in_=dsc[:, :])

    nc.compile()
    return nc


def _bf16(a):
    import concourse.mybir as mybir
    return np.asarray(a).astype(mybir.dt.np(mybir.dt.bfloat16))


def _build_hT(plan, hT_full_f32):
    """Per-core compact hT: [kdim, tot] with per-phase [pad | uniq sources]."""
    kdim = hT_full_f32.shape[0]
    tot = plan["tot"]
    out = []
    for c in range(C):
        hc = np.zeros((kdim, tot), np.float32)
        for p, row0 in enumerate(plan["row0_list"]):
            uniq = plan["uniq_lists"][p][c]
            hc[:, row0 + 1:row0 + 1 + len(uniq)] = hT_full_f32[:, uniq]
        out.append(_bf16(hc))
    return out


def _build_hoT(plan, hT_full_f32):
    kdim = hT_full_f32.shape[0]
    out = []
    for c in range(C):
        own = plan["orders"][c]
        ho = np.zeros((kdim, NSHP), np.float32)
        real = own >= 0
        ho[:, real] = hT_full_f32[:, own[real]]
        out.append(_bf16(ho))
    return out


LAST = {}


def kernel(x, edge_index, W1, a_src1, a_dst1, b1, W2, a_src2, a_dst2, b2):
    from concourse.bass_utils import run_bass_kernel_spmd

    x = np.asarray(x, np.float32)
    edge_index = np.asarray(edge_index)
    W1 = np.asarray(W1, np.float64); a_src1 = np.asarray(a_src1, np.float64)
    a_dst1 = np.asarray(a_dst1, np.float64); b1 = np.asarray(b1, np.float32)
    W2 = np.asarray(W2, np.float64); a_src2 = np.asarray(a_src2, np.float64)
    a_dst2 = np.asarray(a_dst2, np.float64); b2 = np.asarray(b2, np.float32)

    plan = _build_plan(edge_index)
    idx, offs, idxcols = _build_idx_tensor(plan)

    nc1 = _build_launch(IN, plan, offs, idxcols)
    nc2 = _build_launch(OUT, plan, offs, idxcols)

    w1se = _bf16(np.concatenate([W1 * a_src1[None, :], W1], 1))
    w2se = _bf16(np.concatenate([W2 * a_src2[None, :], W2], 1))
    w1ad2 = _bf16(np.stack([W1 @ a_dst1, W1 @ a_src1], 1))
    w2ad2 = _bf16(np.stack([W2 @ a_dst2, W2 @ a_src2], 1))
    rb1 = np.tile(b1, (128, 1)).astype(np.float32)
    rb2 = np.tile(b2, (128, 1)).astype(np.float32)

    xT = np.ascontiguousarray(x.T)
    hTs1 = _build_hT(plan, xT)
    hoTs1 = _build_hoT(plan, xT)
    in_maps1 = [{"hT": hTs1[c], "hoT": hoTs1[c], "wse": w1se, "wad2": w1ad2,
                 "rb": rb1, "idx": idx[c]} for c in range(C)]

    res1 = run_bass_kernel_spmd(nc1, in_maps1, core_ids=list(range(C)))
    LAST["res1"] = res1

    # assemble full node-indexed h2
    h2 = np.zeros((N, OUT), np.float32)
    for c in range(C):
        sh = np.asarray(res1.results[c]["outp"])
        own = plan["orders"][c]
        real = own >= 0
        h2[own[real]] = sh[real]
    h2T = np.ascontiguousarray(h2.T)

    hTs2 = _build_hT(plan, h2T)
    hoTs2 = _build_hoT(plan, h2T)
    in_maps2 = [{"hT": hTs2[c], "hoT": hoTs2[c], "wse": w2se, "wad2": w2ad2,
                 "rb": rb2, "idx": idx[c]} for c in range(C)]

    res2 = run_bass_kernel_spmd(nc2, in_maps2, core_ids=list(range(C)))
    LAST["res2"] = res2

    out = np.empty((N, OUT), np.float32)
    for c in range(C):
        sh = np.asarray(res2.results[c]["outp"])
        own = plan["orders"][c]
        real = own >= 0
        out[own[real]] = sh[real]
    return out


# revision 25
# speedup vs baseline: 1.3943x; 1.2803x over previous
"""Two-layer single-head GAT (GATConv x2) on 8 trn2 NeuronCores — v2.

Strategy: 1D node partition across 8 cores by destination node; edges live
with their destination owner; weights replicated.  The dominant cost is the
per-edge dma_gather (Q7 descriptor generation at ~8ns/index), so the design
minimizes gather indices:

  * Self-loops are handled analytically from the resident hoT (own-dst
    features), removing one slot per destination.
  * Destinations are degree-sorted and processed in tiles of 128; each tile
    uses K_t = max in-tile degree slots (single logical slot space).
  * int16 gather indices only reach 32768 rows, so tiles are grouped into
    PHASES: each phase has a compact per-core table holding only the unique
    sources of that phase's edges (< 32768 rows) => a single gather window
    per tile, no lo/hi split padding (~103k slots/core/layer vs 155k).

Per layer, per core (one SPMD launch per layer):
  Stage A (dense): stream hT (bf16) in big DMAs; per 128-row chunk
    matmul -> PSUM [128, 65] = [H = h@W | as = h@(W a_src)];
    copy H (bf16) + as (fp32, packed in bytes 128..131) into 256-byte table
    rows in DRAM.  Per phase a pad row 0 is overwritten with bf16(-3.39e38)
    so padded slots produce exp(score) == 0.
  Own-dst data: ad/as_own ([128,2] per tile) and h_own ([128,64] per tile)
    from hoT matmuls.
  Stage B: per tile, one dma_gather (256B bf16 rows [H|as|junk]) ->
    scores = lrelu(as + ad), p = exp (fused denominator), U = sum p*H,
    plus the analytic self-loop term, out = (U + p_self*h_own)/den + b.

Layer 1 -> layer 2 crosses cores, so the layers run as two SPMD launches;
the host concatenates the layer-1 shard outputs in between.
"""

import sys

sys.path.insert(0, "/opt/trn_rl_repo")

import numpy as np

N = 50000
E = 800000
IN = 128
OUT = 64
C = 8                      # cores
NSH = N // C               # 6250 dsts per core
NTILES = (NSH + 127) // 128  # 49
NSHP = NTILES * 128        # 6272 padded dsts per core
NEG_SLOPE = 0.2
PHASE_LIMIT = 32000        # max unique sources per (core, phase)
# pad-row element: sum of 64 of these stays finite (~-6.4e37) and exp -> 0
PAD_BF16 = -1.0e36


def _build_plan(edge_index):
    """Host-side graph preprocessing shared by both layers.

    Returns dict with: orders, K_t, phase bounds/rows, per-edge compact ids,
    and per-(core,phase) unique source node lists.
    """
    src = np.asarray(edge_index[0], dtype=np.int64)
    dst = np.asarray(edge_index[1], dtype=np.int64)
    core_of = dst // NSH

    deg_all = np.bincount(dst, minlength=N)  # real in-degree (no self loop)
    pos_of = np.empty(N, dtype=np.int64)
    orders = []
    for c in range(C):
        d0 = c * NSH
        order = np.argsort(-deg_all[d0:d0 + NSH], kind="stable")
        pos_of[d0 + order] = np.arange(NSH)
        orders.append(np.concatenate([order + d0, np.full(NSHP - NSH, -1, np.int64)]))

    epos = pos_of[dst]
    etile = epos // 128

    degpt = np.zeros((C, NSHP), np.int64)
    np.add.at(degpt, (core_of, epos), 1)
    K_t = degpt.reshape(C, NTILES, 128).max(axis=(0, 2))
    K_t = np.maximum(K_t, 1)

    # slot of each edge within its (core, dst)
    okey = np.lexsort((epos, core_of))
    gid = core_of[okey] * NSHP + epos[okey]
    first = np.r_[True, gid[1:] != gid[:-1]]
    lin = np.arange(len(gid))
    start = np.maximum.accumulate(np.where(first, lin, 0))
    slot = np.empty(E, np.int64)
    slot[okey] = lin - start

    # phase boundaries (shared across cores): first phases are small so their
    # tables finish early and the gather stream starts almost immediately;
    # later phases are greedy-maximal under PHASE_LIMIT unique sources.
    caps = [2, 8]
    bounds = []
    t0 = 0
    pi = 0
    while t0 < NTILES:
        cap = caps[pi] if pi < len(caps) else NTILES
        t1 = t0 + 1
        while t1 < NTILES and (t1 - t0) < cap:
            ok = True
            for c in range(C):
                m = (core_of == c) & (etile >= t0) & (etile <= t1)
                if len(np.unique(src[m])) > PHASE_LIMIT:
                    ok = False
                    break
            if not ok:
                break
            t1 += 1
        bounds.append((t0, t1))
        t0 = t1
        pi += 1

    compact = np.zeros(E, np.int64)   # 1-based id within the edge's phase window
    uniq_lists = []                   # [phase][core] -> node ids
    R_list, row0_list = [], []
    row0 = 0
    phase_of_tile = np.zeros(NTILES, np.int64)
    for p, (ta, tb) in enumerate(bounds):
        phase_of_tile[ta:tb] = p
        us = []
        umax = 0
        for c in range(C):
            m = (core_of == c) & (etile >= ta) & (etile < tb)
            uniq, inv = np.unique(src[m], return_inverse=True)
            compact[m] = inv + 1
            us.append(uniq)
            umax = max(umax, len(uniq))
        uniq_lists.append(us)
        # round to 2048 (= GRP chunks of 128) so the stage-A group write
        # permutation (see _phys_row) stays within a phase
        R = ((umax + 1 + 2047) // 2048) * 2048
        assert R <= 32768
        R_list.append(R)
        row0_list.append(row0)
        row0 += R
    tot = row0

    return dict(orders=orders, K_t=K_t, bounds=bounds, R_list=R_list,
                row0_list=row0_list, tot=tot, uniq_lists=uniq_lists,
                phase_of_tile=phase_of_tile, core_of=core_of, epos=epos,
                etile=etile, slot=slot, compact=compact)


GRP = 16  # stage-A chunks (of 128 rows) per group DMA


def _phys_row(lp):
    """Logical phase-relative row -> physical table row.

    Stage A writes a [128, GRP, 128] staging tile to 2048 consecutive table
    rows; the DMA pairs elements in AP order, which lands logical row
    128*j + p (chunk j, partition p) at physical row p*GRP + j within the
    group. Indices must address physical rows.
    """
    lp = np.asarray(lp)
    base = (lp // (128 * GRP)) * (128 * GRP)
    off = lp - base
    return base + (off % 128) * GRP + off // 128


def _wrap_idx(arr):
    """[K,128] slot-major idx array -> [128, 8K] wrapped+replicated int16."""
    flat = arr.reshape(-1)                       # i = k*128 + p
    w = flat.reshape(-1, 16).T                   # [16, NI/16]
    return np.tile(w, (8, 1)).astype(np.int16)


def _build_idx_tensor(plan):
    """Per-core [128, IDXCOLS] int16 idx tensors + per-tile offsets."""
    K_t = plan["K_t"]
    offs = []
    off = 0
    for t in range(NTILES):
        offs.append(off)
        off += 8 * int(K_t[t])
    idxcols = off
    out = np.zeros((C, 128, idxcols), np.int16)
    core_of, epos, etile = plan["core_of"], plan["etile"], plan["etile"]
    epos = plan["epos"]
    slot, compact = plan["slot"], plan["compact"]
    for c in range(C):
        mc = core_of == c
        for t in range(NTILES):
            m = mc & (etile == t)
            arr = np.zeros((int(K_t[t]), 128), np.int64)   # pad -> row 0
            arr[slot[m], epos[m] % 128] = _phys_row(compact[m])
            out[c, :, offs[t]:offs[t] + 8 * int(K_t[t])] = _wrap_idx(arr)
    return out, offs, idxcols


def _build_launch(kdim, plan, offs, idxcols, dbg_tile=None):
    """One SPMD launch: Stage A (compact tables) + Stage B (49 dst tiles)."""
    import concourse.bacc as bacc
    import concourse.mybir as mybir
    from concourse.tile import TileContext

    f32 = mybir.dt.float32
    bf16 = mybir.dt.bfloat16
    K_t = plan["K_t"]
    R_list, row0_list = plan["R_list"], plan["row0_list"]
    phase_of = plan["phase_of_tile"]
    tot = plan["tot"]
    nchunk = tot // 128

    nc = bacc.Bacc(None, target_bir_lowering=False, debug=True)
    hT = nc.declare_dram_parameter("hT", [kdim, tot], bf16, isOutput=False)
    hoT = nc.declare_dram_parameter("hoT", [kdim, NSHP], bf16, isOutput=False)
    wse = nc.declare_dram_parameter("wse", [kdim, 128], bf16, isOutput=False)
    wad2 = nc.declare_dram_parameter("wad2", [kdim, 2], bf16, isOutput=False)
    rb = nc.declare_dram_parameter("rb", [128, 64], f32, isOutput=False)
    idx = nc.declare_dram_parameter("idx", [128, idxcols], mybir.dt.int16, isOutput=False)
    outp = nc.declare_dram_parameter("outp", [NSHP, 64], f32, isOutput=True)
    # one table tensor per phase so a phase's gathers only depend on its writes
    tabls = [nc.dram_tensor(f"tabl{p}", [R, 128], bf16) for p, R in enumerate(R_list)]
    assert tot % (128 * GRP) == 0
    if dbg_tile is not None:
        kdbg = int(K_t[dbg_tile])
        dbg_tg = nc.declare_dram_parameter("dbg_tg", [128, kdbg, 128], bf16, isOutput=True)
        dbg_sc = nc.declare_dram_parameter("dbg_sc", [128, 3 * kdbg + 200], f32, isOutput=True)

    with TileContext(nc) as tc:
        with (
            tc.tile_pool(name="const", bufs=1) as cpool,
            tc.tile_pool(name="io", bufs=2) as io,
            tc.tile_pool(name="acopy", bufs=2) as acopy,
            tc.tile_pool(name="work", bufs=8) as work,
            tc.tile_pool(name="small", bufs=3) as small,
            tc.tile_pool(name="psA", bufs=3, space="PSUM") as psA,
            tc.tile_pool(name="psB", bufs=2, space="PSUM") as psB,
            tc.tile_pool(name="psH", bufs=2, space="PSUM") as psH,
        ):
            wse_sb = cpool.tile([kdim, 128], bf16)
            nc.sync.dma_start(out=wse_sb[:, :], in_=wse[:, :])
            wad2_sb = cpool.tile([kdim, 2], bf16)
            nc.sync.dma_start(out=wad2_sb[:, :], in_=wad2[:, :])
            rb_sb = cpool.tile([128, 64], f32)
            nc.sync.dma_start(out=rb_sb[:, :], in_=rb[:, :])
            idx_sb = cpool.tile([128, idxcols], mybir.dt.int16)
            nc.sync.dma_start(out=idx_sb[:, :], in_=idx[:, :])
            hoT_sb = cpool.tile([kdim, NSHP], bf16)
            nc.sync.dma_start(out=hoT_sb[:, :], in_=hoT[:, :])
            padrow = cpool.tile([1, 128], bf16)
            nc.vector.memset(padrow[:, :], PAD_BF16)
            adas_sb = cpool.tile([128, NTILES, 2], f32)
            ho_all = cpool.tile([128, NTILES, 64], f32)

            # Stage A: table rows [S = h@(W*a_src) (64) | H = h@W (64)] bf16
            for p, (row0, R) in enumerate(zip(row0_list, R_list)):
                for g in range(R // (128 * GRP)):
                    ch0 = row0 // 128 + g * GRP
                    hg = io.tile([kdim, GRP * 128], bf16, tag="hg")
                    nc.sync.dma_start(out=hg[:, :], in_=hT[:, 128 * ch0:128 * (ch0 + GRP)])
                    st = acopy.tile([128, GRP, 128], bf16, tag="st")
                    for j4 in range(GRP // 4):
                        ps = psA.tile([128, 4, 128], f32)
                        for jj in range(4):
                            j = j4 * 4 + jj
                            nc.tensor.matmul(ps[:, jj, :], hg[:, 128 * j:128 * (j + 1)],
                                             wse_sb[:, :], start=True, stop=True)
                        nc.scalar.copy(st[:, 4 * j4:4 * (j4 + 1), :], ps[:, :, :])
                    r0 = g * GRP * 128
                    nc.sync.dma_start(out=tabls[p][r0:r0 + GRP * 128, :], in_=st[:, :, :])
                # pad row AFTER this phase's chunk writes
                nc.sync.dma_start(out=tabls[p][0:1, :], in_=padrow[0:1, :])

            # own-dst data: [ad | as_own] and h_own per tile
            for t in range(NTILES):
                hsl = hoT_sb[:, 128 * t:128 * (t + 1)]
                ps2 = psB.tile([128, 2], f32)
                nc.tensor.matmul(ps2[:, :], hsl, wad2_sb[:, :], start=True, stop=True)
                nc.scalar.copy(adas_sb[:, t, :], ps2[:, :])
                psh = psH.tile([128, 64], f32)
                nc.tensor.matmul(psh[:, :], hsl, wse_sb[:, 64:128], start=True, stop=True)
                nc.scalar.copy(ho_all[:, t, :], psh[:, :])

            # Stage B: one 128-dst tile at a time
            for t in range(NTILES):
                k = int(K_t[t])
                p = int(phase_of[t])
                row0, R = row0_list[p], R_list[p]
                tg = work.tile([128, k, 128], bf16, tag="tg")
                nc.gpsimd.dma_gather(tg[:, :, :], tabls[p][0:R, :],
                                     idx_sb[:, offs[t]:offs[t] + 8 * k],
                                     128 * k, 128 * k, 128, single_packet=False)
                as_t = small.tile([128, k], f32, tag="as")
                nc.vector.tensor_reduce(as_t[:, :], tg[:, :, 0:64], mybir.AxisListType.X,
                                        mybir.AluOpType.add)
                z_t = small.tile([128, k], f32, tag="z")
                nc.vector.tensor_scalar(z_t[:, :], as_t[:, :], adas_sb[:, t, 0:1], None,
                                        mybir.AluOpType.add)
                s_t = small.tile([128, k], f32, tag="s")
                nc.vector.scalar_tensor_tensor(s_t[:, :], z_t[:, :], NEG_SLOPE, z_t[:, :],
                                               mybir.AluOpType.mult, mybir.AluOpType.max)
                p_t = small.tile([128, k], bf16, tag="p")
                den = small.tile([128, 1], f32, tag="den")
                nc.scalar.activation(p_t[:, :], s_t[:, :], mybir.ActivationFunctionType.Exp,
                                     accum_out=den[:, :])
                # self-loop score: s2 = lrelu(as_own + ad)
                z2 = small.tile([128, 1], f32, tag="z2")
                nc.vector.tensor_tensor(z2[:, :], adas_sb[:, t, 1:2], adas_sb[:, t, 0:1],
                                        mybir.AluOpType.add)
                s2 = small.tile([128, 1], f32, tag="s2")
                nc.vector.scalar_tensor_tensor(s2[:, :], z2[:, :], NEG_SLOPE, z2[:, :],
                                               mybir.AluOpType.mult, mybir.AluOpType.max)
                p2 = small.tile([128, 1], f32, tag="p2")
                nc.scalar.activation(p2[:, :], s2[:, :], mybir.ActivationFunctionType.Exp)
                den2 = small.tile([128, 1], f32, tag="den2")
                nc.vector.tensor_tensor(den2[:, :], den[:, :], p2[:, :], mybir.AluOpType.add)
                # weighted aggregation over the H half
                pt = work.tile([128, k, 64], bf16, tag="pt")
                p_b = p_t[:, :].unsqueeze(2).broadcast_to([128, k, 64])
                nc.vector.tensor_tensor(pt[:, :, :], tg[:, :, 64:128], p_b, mybir.AluOpType.mult)
                u = small.tile([128, 64], f32, tag="u")
                nc.vector.tensor_reduce(u[:, :], pt[:, :, :].transpose([0, 2, 1]),
                                        mybir.AxisListType.X, mybir.AluOpType.add)
                ho_p = small.tile([128, 64], f32, tag="ho_p")
                nc.scalar.activation(ho_p[:, :], ho_all[:, t, :],
                                     mybir.ActivationFunctionType.Copy, scale=p2[:, :])
                u2 = small.tile([128, 64], f32, tag="u2")
                nc.vector.tensor_tensor(u2[:, :], ho_p[:, :], u[:, :], mybir.AluOpType.add)
                rd = small.tile([128, 1], f32, tag="rd")
                nc.vector.reciprocal(rd[:, :], den2[:, :])
                o1 = small.tile([128, 64], f32, tag="o1")
                nc.scalar.activation(o1[:, :], u2[:, :],
                                     mybir.ActivationFunctionType.Copy, scale=rd[:, :])
                o = small.tile([128, 64], f32, tag="o")
                nc.vector.tensor_tensor(o[:, :], o1[:, :], rb_sb[:, :], mybir.AluOpType.add)
                nc.sync.dma_start(out=outp[128 * t:128 * (t + 1), :], in_=o[:, :])
                if dbg_tile == t:
                    nc.sync.dma_start(out=dbg_tg[:, :, :], in_=tg[:, :, :])
                    dsc = small.tile([128, 3 * k + 200], f32, tag="dsc")
                    nc.vector.tensor_copy(dsc[:, 0:k], as_t[:, :])
                    nc.vector.tensor_copy(dsc[:, k:2 * k], s_t[:, :])
                    nc.vector.tensor_copy(dsc[:, 2 * k:3 * k], p_t[:, :])
                    base = 3 * k
                    nc.vector.tensor_copy(dsc[:, base:base + 1], den[:, :])
                    nc.vector.tensor_copy(dsc[:, base + 1:base + 2], den2[:, :])
                    nc.vector.tensor_copy(dsc[:, base + 2:base + 3], p2[:, :])
                    nc.vector.tensor_copy(dsc[:, base + 3:base + 4], s2[:, :])
                    nc.vector.tensor_copy(dsc[:, base + 4:base + 68], u[:, :])
                    nc.vector.tensor_copy(dsc[:, base + 68:base + 132], u2[:, :])
                    nc.vector.tensor_copy(dsc[:, base + 132:base + 134], adas_sb[:, t, :])
                    nc.vector.tensor_copy(dsc[:, base + 134:base + 198], ho_all[:, t, :])
                    nc.sync.dma_start(out=dbg_sc[:, :], in_=dsc[:, :])

    nc.compile()
    return nc


def _bf16(a):
    import concourse.mybir as mybir
    return np.asarray(a).astype(mybir.dt.np(mybir.dt.bfloat16))


def _build_hT(plan, hT_full_f32):
    """Per-core compact hT: [kdim, tot] with per-phase [pad | uniq sources]."""
    kdim = hT_full_f32.shape[0]
    tot = plan["tot"]
    out = []
    for c in range(C):
        hc = np.zeros((kdim, tot), np.float32)
        for p, row0 in enumerate(plan["row0_list"]):
            uniq = plan["uniq_lists"][p][c]
            hc[:, row0 + 1:row0 + 1 + len(uniq)] = hT_full_f32[:, uniq]
        out.append(_bf16(hc))
    return out


def _build_hoT(plan, hT_full_f32):
    kdim = hT_full_f32.shape[0]
    out = []
    for c in range(C):
        own = plan["orders"][c]
        ho = np.zeros((kdim, NSHP), np.float32)
        real = own >= 0
        ho[:, real] = hT_full_f32[:, own[real]]
        out.append(_bf16(ho))
    return out


LAST = {}


def kernel(x, edge_index, W1, a_src1, a_dst1, b1, W2, a_src2, a_dst2, b2):
    from concourse.bass_utils import run_bass_kernel_spmd

    x = np.asarray(x, np.float32)
    edge_index = np.asarray(edge_index)
    W1 = np.asarray(W1, np.float64); a_src1 = np.asarray(a_src1, np.float64)
    a_dst1 = np.asarray(a_dst1, np.float64); b1 = np.asarray(b1, np.float32)
    W2 = np.asarray(W2, np.float64); a_src2 = np.asarray(a_src2, np.float64)
    a_dst2 = np.asarray(a_dst2, np.float64); b2 = np.asarray(b2, np.float32)

    plan = _build_plan(edge_index)
    idx, offs, idxcols = _build_idx_tensor(plan)

    nc1 = _build_launch(IN, plan, offs, idxcols)
    nc2 = _build_launch(OUT, plan, offs, idxcols)

    w1se = _bf16(np.concatenate([W1 * a_src1[None, :], W1], 1))
    w2se = _bf16(np.concatenate([W2 * a_src2[None, :], W2], 1))
    w1ad2 = _bf16(np.stack([W1 @ a_dst1, W1 @ a_src1], 1))
    w2ad2 = _bf16(np.stack([W2 @ a_dst2, W2 @ a_src2], 1))
    rb1 = np.tile(b1, (128, 1)).astype(np.float32)
    rb2 = np.tile(b2, (128, 1)).astype(np.float32)

    xT = np.ascontiguousarray(x.T)
    hTs1 = _build_hT(plan, xT)
    hoTs1 = _build_hoT(plan, xT)
    in_maps1 = [{"hT": hTs1[c], "hoT": hoTs1[c], "wse": w1se, "wad2": w1ad2,
                 "rb": rb1, "idx": idx[c]} for c in range(C)]

    res1 = run_bass_kernel_spmd(nc1, in_maps1, core_ids=list(range(C)))
    LAST["res1"] = res1

    # assemble full node-indexed h2
    h2 = np.zeros((N, OUT), np.float32)
    for c in range(C):
        sh = np.asarray(res1.results[c]["outp"])
        own = plan["orders"][c]
        real = own >= 0
        h2[own[real]] = sh[real]
    h2T = np.ascontiguousarray(h2.T)

    hTs2 = _build_hT(plan, h2T)
    hoTs2 = _build_hoT(plan, h2T)
    in_maps2 = [{"hT": hTs2[c], "hoT": hoTs2[c], "wse": w2se, "wad2": w2ad2,
                 "rb": rb2, "idx": idx[c]} for c in range(C)]

    res2 = run_bass_kernel_spmd(nc2, in_maps2, core_ids=list(range(C)))
    LAST["res2"] = res2

    out = np.empty((N, OUT), np.float32)
    for c in range(C):
        sh = np.asarray(res2.results[c]["outp"])
        own = plan["orders"][c]
        real = own >= 0
        out[own[real]] = sh[real]
    return out
